# revision 1
# baseline (speedup 1.0000x reference)
"""Trainium2 Bass kernel for nn_AttentionModel — transposed-gates layout (v2).

Math refactor (exact, weight-level), as v1:
    feats = images @ W_fc.T + b_fc
    ctx_t = [feats, emb_t] @ W_att.T + b_att
    gates = [feats, ctx_t] @ W_ih.T + b_ih + h @ W_hh.T + b_hh
        = gx0 + emb_t @ W_att_e.T @ W_ih_c.T + h @ W_hh.T
    gx0   = feats @ W1 + b_eff                      (constant over t)

v2 layout: gates are computed TRANSPOSED — gate-dim on partitions, batch on
the free dim.  The CoreSim cost model prices a matmul at out-free-size
cycles regardless of partition count, so [128 gd, 32 b] outputs cost 32
cycles where v1's [32 b, 256 gd] col-tiled outputs cost 256.  The gate dim
(2048) is split into 16 chunks of 128, ordered (half, gate, kchunk) with
gates (i, f, o, g):
    c = h*8 + gt*2 + kch   ->  hidden unit  (gt, h*256 + kch*128 + p)
Each hidden-half h gets its OWN one-bank PSUM tile [128, 8, 32] so the
sigmoid read of half A never blocks half B's accumulating matmuls (the
sim's dep tracking is coarser than chunk slices).  Within a tile, a single
full-tile identity-matmul seed (start=True) pulls in gx0T and clears the
bank's pending-zero bytes, so all later accumulates are order-independent
(skip_group_check bypasses the region-level group check; per-element
has_written semantics make this exact on HW too).

h_t comes out of the cell directly in hT layout [128 hid-part, 4 kc, 32 b]
— no PE transposes.  The cell chain runs on Pool (gpsimd) in program order
to avoid per-hop semaphore delays; only sigmoid/tanh sit on ScalarE.
"""

import sys

sys.path.insert(0, "/opt/trn_rl_repo")

import numpy as np
import ml_dtypes

FP8 = None  # set below after mybir import

import concourse.bass as bass
import concourse.tile as tile
from concourse import bacc, mybir

BF16 = mybir.dt.bfloat16
F32 = mybir.dt.float32
FP8 = mybir.dt.float8e4
SC = 16.0  # fp8 scaling: weights x16, h stored as 16*h -> gates x256

B, T = 256, 128
EMBED, HIDDEN, VOCAB, FC_IN = 256, 512, 1004, 2048
NCORES = 8
BS = B // NCORES  # 32 batch rows per core
G4 = 4 * HIDDEN  # 2048 gate dim
NCH = 16  # gate-dim chunks of 128
VOCAB_TILES = 8  # 7*128 + 108
NT_CHUNKS = 8  # time chunks of 16 steps for phase C

# target gate order (i, f, o, g); source order is (i, f, g, o)
GATE_SRC = (0, 1, 3, 2)


def _bf16(x):
    return np.ascontiguousarray(x.astype(ml_dtypes.bfloat16))


def _fp8(x):
    return np.ascontiguousarray(x.astype(ml_dtypes.float8_e4m3))


def _f32(x):
    return np.ascontiguousarray(x.astype(np.float32))


def _chunk_col_ranges():
    """16 (src_col_base, scale) for chunk order c = h*8 + gt*2 + kch."""
    out = []
    for h in range(2):
        for gt in range(4):
            sg = GATE_SRC[gt]
            for kch in range(2):
                out.append((sg * HIDDEN + h * 256 + kch * 128,
                            2.0 if gt == 3 else 1.0))
    return out


def _gchunks(w):
    """w[..., 2048] in source (i,f,g,o) order -> [..., 16, 128] in device
    chunk order, with the g gate pre-scaled by 2 (tanh(x)=2*sigmoid(2x)-1)."""
    cols = [w[..., b:b + 128] * s for (b, s) in _chunk_col_ranges()]
    return np.stack(cols, axis=-2)


def _beff97(b):
    """[2048] -> [33, 1024]: rows 0/32 hold 8 chunks each (legal lhsT base
    partitions; 2KB on two partitions keeps the DMA cheap)."""
    out = np.zeros((33, 1024), b.dtype)
    out[0, :] = b[:1024]
    out[32, :] = b[1024:]
    return out


def _part_fold(w, kdim):
    """[K, N...] -> [128, K//128, N...] so partition p holds rows {c*128+p}."""
    k = w.shape[0]
    assert k == kdim and k % 128 == 0
    return np.ascontiguousarray(
        w.reshape(k // 128, 128, *w.shape[1:]).transpose(
            1, 0, *range(2, w.ndim + 1)))


def prepare_host(images, captions, W_fc, b_fc, W_att, b_att,
                 W_ih, b_ih, W_hh, b_hh, W_out, b_out):
    f64 = np.float64
    W_att_f = W_att[:, :EMBED].astype(f64)     # [256, 256]
    W_att_e = W_att[:, EMBED:].astype(f64)     # [256, 512]
    W_ih_f = W_ih[:, :EMBED].astype(f64)       # [2048, 256]
    W_ih_c = W_ih[:, EMBED:].astype(f64)       # [2048, 256]

    W1 = W_ih_f.T + W_att_f.T @ W_ih_c.T       # [256, 2048]
    b_eff = W_ih_c @ b_att.astype(f64) + b_ih.astype(f64) + b_hh.astype(f64)

    # device chunk order (+ g-gate x2)
    w1c = _gchunks(W1).reshape(EMBED, G4)                    # [256, 2048']
    beffc = _gchunks(b_eff[None, :])[0].reshape(G4)          # [2048']
    wihc_l = _part_fold(_gchunks(np.ascontiguousarray(W_ih_c.T)),
                        EMBED).reshape(128, 2, G4)           # [128,2,2048']
    whh_l = _part_fold(_gchunks(W_hh.T.astype(f64)),
                       HIDDEN).reshape(128, 4, G4)           # [128,4,2048']

    shared = {
        "wfc0": _bf16(np.ascontiguousarray(
            _part_fold(W_fc.T.astype(f64), FC_IN)[:, :, :128]
        ).reshape(128, 16 * 128)),                                # [128,2048]
        "wfc1": _bf16(np.ascontiguousarray(
            _part_fold(W_fc.T.astype(f64), FC_IN)[:, :, 128:]
        ).reshape(128, 16 * 128)),                                # [128,2048]
        "w1c": _bf16(_part_fold(w1c, EMBED) * SC * SC),           # [128,2,2048] x256
        "watte": _bf16(_part_fold(np.ascontiguousarray(W_att_e.T), HIDDEN)),
        "wihc": _bf16(wihc_l * SC * SC),                          # [128,2,2048] x256
        "whh8": _fp8(whh_l * SC),                                 # [128,4,2048] fp8 x16
        "wout": _bf16(_part_fold(np.ascontiguousarray(W_out.T.astype(f64)),
                                 HIDDEN)),                        # [128,4,1004]
        "beffc": _bf16(_beff97(beffc * SC * SC)),                 # [97,512] x256
        "bfc": _f32(b_fc.reshape(2, 128).T.copy()),               # [128,2]
        "bout": _f32(np.concatenate([b_out, np.zeros(1024 - VOCAB,
                                                     b_out.dtype)]
                                    ).reshape(8, 128).T.copy()),  # [128,8]
        "ones32": _bf16(np.ones((33, BS))),                       # [33,32]
        "id128": _bf16(np.eye(128)),                              # [128,128]
    }

    in_maps = []
    for c in range(NCORES):
        sl = slice(c * BS, (c + 1) * BS)
        img_t = images[sl].T.astype(f64)                          # [2048, 32]
        caps = captions[sl]                                       # [32,T,512]
        # capsT4[tq] flat [128, 512]: (p, j*128 + kc*32 + b) =
        # caps[b, 4tq+j, kc*128+p]; flat layout keeps the DMA innermost
        # contiguous run at 1 KB (full-rate DMA in the cost model).
        capsT = caps.transpose(1, 2, 0).reshape(T, 4, 128, BS)
        capsT = capsT.transpose(0, 2, 1, 3)                       # [T,128,4,32]
        capsT = capsT.reshape(T // 4, 4, 128, 4, BS)
        capsT = capsT.transpose(0, 2, 1, 3, 4).reshape(T // 4, 128, 4 * 4 * BS)
        m = dict(shared)
        m["imagesT"] = _bf16(_part_fold(img_t, FC_IN).reshape(128, 16 * BS))
        m["capsT"] = _bf16(capsT)                                 # [T/4,128,512]
        in_maps.append(m)
    return in_maps


def numpy_forward(m):
    """Simulate the device program in numpy (f32) from one core's input map.

    Returns outT [VOCAB, T*BS] matching d_out."""
    f = np.float32
    wfc = np.concatenate([m["wfc0"].astype(f).reshape(128, 16, 128),
                          m["wfc1"].astype(f).reshape(128, 16, 128)],
                         axis=2)       # [128,16,256]
    imgT = m["imagesT"].astype(f).reshape(128, 16, BS)
    bfc = m["bfc"].astype(f)        # [128,2]
    w1c = m["w1c"].astype(f)        # [128,2,2048]
    b97 = m["beffc"].astype(f)      # [33, 1024]
    beffc = np.concatenate([b97[0], b97[32]])  # [2048]
    watte = m["watte"].astype(f)    # [128,4,256]
    wihc = m["wihc"].astype(f)      # [128,2,2048] (x256)
    whh8 = m["whh8"].astype(f)      # [128,4,2048] fp8 (x16)
    wout = m["wout"].astype(f)      # [128,4,1004]
    bout = m["bout"].astype(f)      # [128,8]
    capsT = m["capsT"].astype(f).reshape(T // 4, 128, 4, 4, BS)

    def r(x):  # bf16 round-trip
        return x.astype(ml_dtypes.bfloat16).astype(f)

    # phase A: featsT [128, 2, 32]
    featsT = np.zeros((128, 2, BS), f)
    for ec in range(2):
        acc = np.zeros((128, BS), f)
        for kc in range(16):
            acc += wfc[:, kc, ec * 128:(ec + 1) * 128].T @ imgT[:, kc, :]
        featsT[:, ec] = acc + bfc[:, ec:ec + 1]
    featsT = r(featsT)

    # phase A2: gx0T [128, 16, 32] (gate-dim chunks on partitions)
    gx0T = np.zeros((128, NCH, BS), f)
    for c in range(NCH):
        acc = np.outer(beffc[c * 128:(c + 1) * 128], np.ones(BS, f))
        for ec in range(2):
            acc += w1c[:, ec, c * 128:(c + 1) * 128].T @ featsT[:, ec, :]
        gx0T[:, c] = acc
    gx0T = r(gx0T)  # beffc/w1c pre-scaled x256 on host

    c_state = np.zeros((128, 4, BS), f)
    hs = np.zeros((128, T, 4, BS), f)          # bf16 h (out-proj)
    hs8 = np.zeros((128, 4, T * BS), f)        # fp8 16*h (recurrence)

    def sig(x):
        return 1.0 / (1.0 + np.exp(-x))

    def fp8c(x):
        return x.astype(ml_dtypes.float8_e4m3).astype(f)

    for t in range(T):
        cap = capsT[t // 4][:, t % 4]          # [128, 4, 32]
        # stage1: ctx_eT [128, 2, 32]
        cx = np.zeros((128, 2, BS), f)
        for ec in range(2):
            acc = np.zeros((128, BS), f)
            for kc in range(4):
                acc += watte[:, kc, ec * 128:(ec + 1) * 128].T @ cap[:, kc, :]
            cx[:, ec] = acc
        cx = r(cx)

        # gates psum: two half tiles [128, 8, 32]
        ps = np.zeros((128, NCH, BS), f)
        for c in range(NCH):
            acc = gx0T[:, c, :].copy()                   # seed
            for ec in range(2):
                acc += wihc[:, ec, c * 128:(c + 1) * 128].T @ cx[:, ec]
            if t > 0:
                for kc in range(4):
                    acc += whh8[:, kc, c * 128:(c + 1) * 128].T @ hs8[
                        :, kc, (t - 1) * BS:t * BS]
            ps[:, c] = acc

        # cell per half
        for h in range(2):
            blk = ps[:, 8 * h:8 * h + 8, :]       # (i,i,f,f,o,o,g,g)
            acts = r(sig(blk / (SC * SC)))        # sigmoid scale=1/256
            tg = r(2.0 * acts[:, 6:8] - 1.0)
            t1 = r(acts[:, 0:2] * tg)
            t2 = r(acts[:, 2:4] * c_state[:, 2 * h:2 * h + 2])
            c_state[:, 2 * h:2 * h + 2] = r(t1 + t2)
            tc = r(np.tanh(c_state[:, 2 * h:2 * h + 2]))
            o16 = r(acts[:, 4:6] * SC)
            hv8 = fp8c(o16 * tc)
            hs8[:, 2 * h, t * BS:(t + 1) * BS] = hv8[:, 0]
            hs8[:, 2 * h + 1, t * BS:(t + 1) * BS] = hv8[:, 1]
            hs[:, t, 2 * h:2 * h + 2] = r(acts[:, 4:6] * tc)

    # phase C: outT [VOCAB, T*BS]
    outT = np.zeros((VOCAB, T * BS), f)
    hsv = r(hs)
    for mt in range(VOCAB_TILES):
        mv = 128 if mt < 7 else VOCAB - 7 * 128
        acc = np.zeros((mv, T * BS), f)
        for kc in range(4):
            rhs = hsv[:, :, kc, :].reshape(128, T * BS)
            acc += wout[:, kc, mt * 128:mt * 128 + mv].T @ rhs
        outT[mt * 128:mt * 128 + mv] = acc + bout[:mv, mt:mt + 1]
    return outT


def build_nc():
    nc = bacc.Bacc("TRN2", target_bir_lowering=False)

    d_imagesT = nc.declare_dram_parameter("imagesT", [128, 16 * BS], BF16, isOutput=False)
    d_capsT = nc.declare_dram_parameter("capsT", [T // 4, 128, 16 * BS], BF16, isOutput=False)
    d_wfc0 = nc.declare_dram_parameter("wfc0", [128, 16 * 128], BF16, isOutput=False)
    d_wfc1 = nc.declare_dram_parameter("wfc1", [128, 16 * 128], BF16, isOutput=False)
    d_w1c = nc.declare_dram_parameter("w1c", [128, 2, G4], BF16, isOutput=False)
    d_watte = nc.declare_dram_parameter("watte", [128, 4, EMBED], BF16, isOutput=False)
    d_wihc = nc.declare_dram_parameter("wihc", [128, 2, G4], BF16, isOutput=False)
    d_whh8 = nc.declare_dram_parameter("whh8", [128, 4, G4], FP8, isOutput=False)
    d_wout = nc.declare_dram_parameter("wout", [128, 4, VOCAB], BF16, isOutput=False)
    d_beffc = nc.declare_dram_parameter("beffc", [33, 1024], BF16, isOutput=False)
    d_bfc = nc.declare_dram_parameter("bfc", [128, 2], F32, isOutput=False)
    d_bout = nc.declare_dram_parameter("bout", [128, 8], F32, isOutput=False)
    d_ones32 = nc.declare_dram_parameter("ones32", [33, BS], BF16, isOutput=False)
    d_id128 = nc.declare_dram_parameter("id128", [128, 128], BF16, isOutput=False)

    d_out = nc.declare_dram_parameter("outT", [VOCAB, T * BS], F32, isOutput=True)

    Sig = mybir.ActivationFunctionType.Sigmoid
    Tanh = mybir.ActivationFunctionType.Tanh

    with tile.TileContext(nc) as tc:
        with (
            tc.tile_pool(name="weights", bufs=1) as wpool,
            tc.tile_pool(name="consts", bufs=1) as cpool,
            tc.tile_pool(name="caps", bufs=4) as cappool,
            tc.tile_pool(name="cell", bufs=4) as cellpool,
            tc.tile_pool(name="psg", bufs=2, space="PSUM") as psg,
            tc.tile_pool(name="psc", bufs=2, space="PSUM") as psc,
            tc.tile_pool(name="pso", bufs=2, space="PSUM") as pso,
            tc.tile_pool(name="outsb", bufs=6) as opool,
        ):
            # --- weights/constants into SBUF (split across DMA queues) ---
            sb_imgT = wpool.tile([128, 16 * BS], BF16)
            sb_wfc0 = wpool.tile([128, 16 * 128], BF16)
            sb_wfc1 = wpool.tile([128, 16 * 128], BF16)
            sb_w1c = wpool.tile([128, 2, G4], BF16)
            sb_watte = wpool.tile([128, 4, EMBED], BF16)
            sb_wihc = wpool.tile([128, 2, G4], BF16)
            sb_whh8 = wpool.tile([128, 4, G4], FP8)
            sb_wout = wpool.tile([128, 4, VOCAB], BF16)
            sb_beffc = cpool.tile([33, 1024], BF16)
            sb_ones32 = cpool.tile([33, BS], BF16)
            sb_bfc = cpool.tile([128, 2], F32)
            sb_bout = cpool.tile([128, 8], F32)
            sb_id128 = cpool.tile([128, 128], BF16)
            def emit_caps_dma2(tq, eng):
                sb_cap4 = cappool.tile([128, 16 * BS], BF16, tag="cap")
                eng.dma_start(out=sb_cap4[:], in_=d_capsT[tq])
                return sb_cap4

            def emit_caps_dma(tq):
                return emit_caps_dma2(tq, nc.sync)

            # phase-A critical tensors first on SP; caps right behind
            for dst, src in [(sb_imgT, d_imagesT), (sb_wfc0, d_wfc0),
                             (sb_bfc, d_bfc), (sb_ones32, d_ones32)]:
                nc.sync.dma_start(out=dst[:], in_=src[:])
            cap_bufs = {0: emit_caps_dma(0), 1: emit_caps_dma(1)}
            # step-0 tensors on scalar queue (wfc1 first: phase A ec=1)
            for dst, src in [(sb_wfc1, d_wfc1), (sb_watte, d_watte),
                             (sb_id128, d_id128), (sb_wihc, d_wihc)]:
                nc.scalar.dma_start(out=dst[:], in_=src[:])
            # A2 weights early on gpsimd; whh8 needed t=1, wout t>=15
            for dst, src in [(sb_w1c, d_w1c), (sb_beffc, d_beffc),
                             (sb_whh8, d_whh8), (sb_wout, d_wout),
                             (sb_bout, d_bout)]:
                nc.gpsimd.dma_start(out=dst[:], in_=src[:])

            # --- phase A: featsT [128, 2, 32] ---
            sb_featsT = cpool.tile([128, 2, BS], BF16)
            for ec in range(2):
                sb_wfc_h = sb_wfc0 if ec == 0 else sb_wfc1
                ps_f = psg.tile([128, 8, BS], F32, tag="gA")
                for kc in range(16):
                    nc.tensor.matmul(
                        ps_f[:, 0, :],
                        lhsT=sb_wfc_h[:, kc * 128:(kc + 1) * 128],
                        rhs=sb_imgT[:, kc * BS:(kc + 1) * BS],
                        start=(kc == 0), stop=(kc == 15),
                    )
                nc.vector.tensor_add(
                    sb_featsT[:, ec, :], ps_f[:, 0, :],
                    sb_bfc[:, ec:ec + 1].broadcast_to((128, BS)),
                )

            # --- phase A2: gx0T [128, 16, 32] = (b_eff + feats @ W1).T ---
            sb_gx0T = cpool.tile([128, NCH, BS], BF16)
            for c in range(NCH):
                ps_x = psg.tile([128, 8, BS], F32, tag="gB")
                bp = 32 * (c // 8)
                nc.tensor.matmul(
                    ps_x[:, 0, :],
                    lhsT=sb_beffc[bp:bp + 1,
                                  (c % 8) * 128:(c % 8 + 1) * 128],
                    rhs=sb_ones32[bp:bp + 1, :],
                    start=True, stop=False,
                )
                for ec in range(2):
                    nc.tensor.matmul(
                        ps_x[:, 0, :],
                        lhsT=sb_w1c[:, ec, c * 128:(c + 1) * 128],
                        rhs=sb_featsT[:, ec, :],
                        start=False, stop=(ec == 1),
                    )
                nc.vector.tensor_copy(sb_gx0T[:, c, :], ps_x[:, 0, :])

            # --- phase B: the 128-step recurrence ---
            sb_hs = wpool.tile([128, T, 4, BS], BF16)
            sb_hs8 = wpool.tile([128, 4, T * BS], FP8)
            sb_c = cpool.tile([128, 4, BS], BF16)
            nc.vector.memset(sb_c[:], 0.0)

            # (tstart, tlen, mt): 8-step spans make 16 units per 16 steps
            # -- exactly the 1-unit-per-step drain rate, so phase C never
            # bursts past the chain-bound per-step PE slack
            spans = [(8 * n, 8) for n in range(15)]
            spans += [(120, 4), (124, 4)]
            out_units = [(ts, tl, mt) for (ts, tl) in spans
                         for mt in range(VOCAB_TILES)]

            def emit_stage1(sb_cap4):
                """ctx_eT for a quad -> sb_cx4 [128, 2(ec), 4(j), 32]"""
                ps_cx = psc.tile([128, 2, 4, BS], F32, tag="cx")
                for ec in range(2):
                    for j in range(4):
                        for kc in range(4):
                            nc.tensor.matmul(
                                ps_cx[:, ec, j, :],
                                lhsT=sb_watte[:, kc, ec * 128:(ec + 1) * 128],
                                rhs=sb_cap4[:, (j * 4 + kc) * BS:
                                            (j * 4 + kc + 1) * BS],
                                start=(kc == 0), stop=(kc == 3),
                            )
                sb_cx4 = cellpool.tile([128, 2, 4, BS], BF16, tag="cx")
                nc.vector.tensor_copy(sb_cx4[:], ps_cx[:])
                return sb_cx4

            def emit_seed_stage2(ps_h, sb_cx4, j, t):
                # per half: one full-tile identity seed (start=True) then
                # order-independent accumulates (see module docstring)
                for h in range(2):
                    nc.tensor.matmul(
                        ps_h[h][:, :, :], lhsT=sb_id128[:, :],
                        rhs=sb_gx0T[:, 8 * h:8 * h + 8, :],
                        start=True, stop=False, skip_group_check=True,
                    )
                for c in range(NCH):
                    for ec in range(2):
                        nc.tensor.matmul(
                            ps_h[c // 8][:, c % 8, :],
                            lhsT=sb_wihc[:, ec, c * 128:(c + 1) * 128],
                            rhs=sb_cx4[:, ec, j, :],
                            start=False, stop=(t == 0 and ec == 1),
                            skip_group_check=True,
                        )

            def emit_rec(ps_h, t, kks, cs):
                # fp8 DoubleRow: one matmul contracts K=256 (h chunks
                # 2kk, 2kk+1) at 0.5 cycles/row
                for c in cs:
                    for kk in kks:
                        nc.tensor.matmul(
                            ps_h[c // 8][:, c % 8, :],
                            lhsT=sb_whh8[:, 2 * kk:2 * kk + 2,
                                         c * 128:(c + 1) * 128],
                            rhs=sb_hs8[:, 2 * kk:2 * kk + 2,
                                       (t - 1) * BS:t * BS],
                            start=False, stop=(kk == 1),
                            skip_group_check=True,
                            perf_mode=mybir.MatmulPerfMode.DoubleRow,
                        )

            def emit_cell_half(ps_h, t, h):
                hsl = slice(2 * h, 2 * h + 2)
                acts = cellpool.tile([128, 8, BS], BF16, tag=f"acts{h}")
                nc.scalar.activation(acts[:], ps_h[h][:, :, :], Sig,
                                     scale=1.0 / (SC * SC))
                tg = cellpool.tile([128, 2, BS], BF16, tag=f"tg{h}")
                nc.gpsimd.tensor_scalar(
                    out=tg[:], in0=acts[:, 6:8, :],
                    scalar1=2.0, scalar2=-1.0,
                    op0=mybir.AluOpType.mult, op1=mybir.AluOpType.add,
                )
                t1 = cellpool.tile([128, 2, BS], BF16, tag=f"t1{h}")
                nc.gpsimd.tensor_mul(t1[:], acts[:, 0:2, :], tg[:])
                t2 = cellpool.tile([128, 2, BS], BF16, tag=f"t2{h}")
                nc.vector.tensor_mul(t2[:], acts[:, 2:4, :], sb_c[:, hsl, :])
                nc.gpsimd.tensor_add(sb_c[:, hsl, :], t1[:], t2[:])
                o16 = cellpool.tile([128, 2, BS], BF16, tag=f"o16{h}")
                nc.gpsimd.tensor_scalar(
                    out=o16[:], in0=acts[:, 4:6, :], scalar1=SC, scalar2=None,
                    op0=mybir.AluOpType.mult,
                )
                tc_t = cellpool.tile([128, 2, BS], BF16, tag=f"tc{h}")
                nc.scalar.activation(tc_t[:], sb_c[:, hsl, :], Tanh)
                nc.gpsimd.tensor_mul(
                    sb_hs8[:, hsl, t * BS:(t + 1) * BS], o16[:], tc_t[:])
                nc.gpsimd.tensor_mul(sb_hs[:, t, hsl, :], acts[:, 4:6, :],
                                     tc_t[:])

            def emit_out_unit(ts, tl, mt):
                tspan = slice(ts, ts + tl)
                nf = tl * BS
                mv = 128 if mt < 7 else VOCAB - 7 * 128
                msl = slice(mt * 128, mt * 128 + mv)
                ps_o = pso.tile([128, 512], F32, tag="o")
                for kc in range(4):
                    nc.tensor.matmul(
                        ps_o[:mv, :nf],
                        lhsT=sb_wout[:, kc, msl],
                        rhs=sb_hs[:, tspan, kc, :],
                        start=(kc == 0), stop=(kc == 3),
                    )
                sb_o = opool.tile([128, 512], F32)
                nc.vector.tensor_scalar(
                    out=sb_o[:mv, :nf], in0=ps_o[:mv, :nf],
                    scalar1=sb_bout[:mv, mt:mt + 1], scalar2=None,
                    op0=mybir.AluOpType.add,
                )
                nc.sync.dma_start(
                    out=d_out[msl, ts * BS:ts * BS + nf],
                    in_=sb_o[:mv, :nf])

            def new_ps_pair():
                ps_hA = psg.tile([128, 8, BS], F32, tag="gA")
                ps_hB = psg.tile([128, 8, BS], F32, tag="gB")
                return (ps_hA, ps_hB)

            # prologue: stage1 quad 0; seed+stage2 for t=0
            cx4_bufs = {0: emit_stage1(cap_bufs[0])}
            pair0 = new_ps_pair()
            ps_gates = {0: pair0}
            emit_seed_stage2(ps_gates[0], cx4_bufs[0], 0, 0)

            for t in range(T):
                ps_h = ps_gates.pop(t)
                if t > 0:
                    emit_rec(ps_h, t, [0], range(8))
                    emit_rec(ps_h, t, [0], range(8, NCH))
                    emit_rec(ps_h, t, [1], range(8))
                emit_cell_half(ps_h, t, 0)
                if t > 0:
                    emit_rec(ps_h, t, [1], range(8, NCH))
                emit_cell_half(ps_h, t, 1)

                # h-independent work for t+1 (fills PE while cell runs)
                if t + 1 < T:
                    tn = t + 1
                    tqn = tn // 4 + 1  # prefetch caps one quad ahead
                    if tn % 4 == 0 and tqn < T // 4:
                        cap_bufs[tqn] = emit_caps_dma(tqn)
                    if tn % 4 == 0:
                        cx4_bufs[tn // 4] = emit_stage1(cap_bufs.pop(tn // 4))
                    sb_cx4 = cx4_bufs[tn // 4]
                    if tn % 4 == 3:
                        cx4_bufs.pop(tn // 4)
                    ps_n = new_ps_pair()
                    ps_gates[tn] = ps_n
                    emit_seed_stage2(ps_n, sb_cx4, tn % 4, tn)

                # phase C interleaved: strictly one unit per step
                avail = sum(1 for (ts, tl, _) in out_units if ts + tl - 1 <= t)
                for _ in range(min(avail, 1)):
                    ts, tl, mt = out_units.pop(0)
                    emit_out_unit(ts, tl, mt)
            while out_units:
                ts, tl, mt = out_units.pop(0)
                emit_out_unit(ts, tl, mt)
    nc.compile()
    return nc


_NC_CACHE = None


def kernel(**inputs) -> np.ndarray:
    global _NC_CACHE
    from concourse.bass_utils import run_bass_kernel_spmd

    in_maps = prepare_host(**inputs)
    if _NC_CACHE is None:
        _NC_CACHE = build_nc()
    nc = _NC_CACHE
    res = run_bass_kernel_spmd(nc, in_maps, list(range(NCORES)))
    outs = []
    for c in range(NCORES):
        o = res.results[c]["outT"]  # [1004, 4096] f32, free = (t, b)
        o = o.reshape(VOCAB, T, BS).transpose(2, 1, 0)  # [32, T, 1004]
        outs.append(o)
    return np.ascontiguousarray(np.concatenate(outs, axis=0).astype(np.float32))


if __name__ == "__main__":
    nc = build_nc()
    print("built ok")

